# revision 34
# baseline (speedup 1.0000x reference)
"""Multi-head self-attention (B=4, T=2048, C=1024, H=16, D=64) on 8 NeuronCores.

Sharding: tensor-parallel over heads (Megatron): each core owns 2 heads.
Wq/Wk/Wv column-sharded, Wo row-sharded; host sums the 8 partial outputs.

Device layout is fully "transposed" (features on partitions, tokens on the
free dim) so that softmax runs over the PSUM free dim and the PV matmul needs
no attention-matrix transpose.

Precision plan: fp16 everywhere except the PV contraction, which runs in
fp8e4 DoubleRow mode (contracting two 128-key tiles per matmul => half the
PE streaming time). To keep accuracy, V is split hi/lo: the 128 stationary
columns of the PV matmul are [v_hi(64) | ones(1) | v_lo(63)], so the PSUM
holds ctx_hi (rows 0-63), the softmax denominator (row 64), and an fp8
error-correction term (rows 65-127) that is added back on the DVE.
exp() output is written as fp8e4 directly by the Scalar engine with a
-ln(16) bias shift to keep values inside the e4m3 range (cancels in the
softmax normalization).

The PE executes its stream in order, so projection/output-projection matmuls
of adjacent batches are interleaved into the attention j-loop (which is
paced by the Scalar engine's exp) to keep both engines near 100% busy.
"""

import numpy as np

import concourse.bass as bass
import concourse.tile as tile
from concourse import bacc, mybir
from concourse.bass_utils import run_bass_kernel_spmd

B, T, C, H, D = 4, 2048, 1024, 16, 64
NCORES = 8
HPC = H // NCORES          # heads per core = 2
F = HPC * D                # per-core feature width = 128
TT = B * T                 # total tokens = 8192

FP32 = mybir.dt.float32
FP16 = mybir.dt.float16
MM_DT = mybir.dt.float16   # matmul compute dtype
PV_DT = mybir.dt.float8e4  # PV matmul dtype (DoubleRow: 2 key-tiles per MM)
OUT_DT = mybir.dt.float16  # partial-output DMA dtype

TILE_K = 128               # contraction tile
TILE_N = 512               # moving free dim per matmul
NK_C = C // TILE_K         # 8 k-tiles over channels
NT4 = T // TILE_N          # 4 token chunks per batch
NJ = T // TILE_K           # 16 key tiles per batch
NP = NJ // 2               # 8 key-tile PAIRS per batch (DoubleRow)
NLO = D - 1                # 63 lo-correction dims (dim 63 is hi-only)

EXP_SHIFT = -2.772588722239781   # -ln(16): keep exp() inside e4m3 range


def build_kernel_body(tc):
    nc = tc.nc
    Exp = mybir.ActivationFunctionType.Exp
    DR = mybir.MatmulPerfMode.DoubleRow

    xT = nc.dram_tensor("xT", [C, TT], MM_DT, kind="ExternalInput").ap()
    wq = nc.dram_tensor("wq", [C, F], MM_DT, kind="ExternalInput").ap()
    wk = nc.dram_tensor("wk", [C, F], MM_DT, kind="ExternalInput").ap()
    wv = nc.dram_tensor("wv", [C, F], MM_DT, kind="ExternalInput").ap()
    wo = nc.dram_tensor("wo", [F, C], MM_DT, kind="ExternalInput").ap()
    bqv = nc.dram_tensor("bq", [F], FP32, kind="ExternalInput").ap()
    bkv = nc.dram_tensor("bk", [F], FP32, kind="ExternalInput").ap()
    outT = nc.dram_tensor("outT", [C, TT], OUT_DT, kind="ExternalOutput").ap()

    import contextlib
    ctx = contextlib.ExitStack()
    with ctx:
        consts = ctx.enter_context(tc.tile_pool(name="consts", bufs=1))
        xpool = ctx.enter_context(tc.tile_pool(name="xt", bufs=64))
        bigs = ctx.enter_context(tc.tile_pool(name="bigs", bufs=2))
        epool = ctx.enter_context(tc.tile_pool(name="expp", bufs=6))
        small = ctx.enter_context(tc.tile_pool(name="small", bufs=2))
        vstage = ctx.enter_context(tc.tile_pool(name="vstage", bufs=2))
        ps_qk = ctx.enter_context(tc.tile_pool(name="ps_qk", bufs=2, space="PSUM"))
        ps_pv = ctx.enter_context(tc.tile_pool(name="ps_pv", bufs=1, space="PSUM"))
        ps_aux = ctx.enter_context(tc.tile_pool(name="ps_aux", bufs=2, space="PSUM"))

        # ---- constants ----
        wq_sb = consts.tile([TILE_K, C], MM_DT)  # c-tile k at [:, k*F:(k+1)*F]
        nc.sync.dma_start(
            wq_sb[:].rearrange("p (k f) -> p k f", k=NK_C),
            wq.rearrange("(k p) f -> p k f", p=TILE_K))
        wk_sb = consts.tile([TILE_K, C], MM_DT)
        nc.sync.dma_start(
            wk_sb[:].rearrange("p (k f) -> p k f", k=NK_C),
            wk.rearrange("(k p) f -> p k f", p=TILE_K))
        wv_sb = consts.tile([TILE_K, C], MM_DT)
        nc.sync.dma_start(
            wv_sb[:].rearrange("p (k f) -> p k f", p=TILE_K, k=NK_C),
            wv.rearrange("(k p) f -> p k f", p=TILE_K))
        wo_sb = consts.tile([F, C], MM_DT)
        nc.sync.dma_start(wo_sb[:], wo)
        bq_sb = consts.tile([F, 1], FP32)
        nc.sync.dma_start(bq_sb[:], bqv.rearrange("(p one) -> p one", one=1))
        bk_sb = consts.tile([F, 1], FP32)
        nc.sync.dma_start(bk_sb[:], bkv.rearrange("(p one) -> p one", one=1))
        ones8 = consts.tile([128, NJ * HPC], PV_DT)
        nc.gpsimd.memset(ones8[:], 1.0)
        nln16 = consts.tile([128, 1], FP32)
        nc.gpsimd.memset(nln16[:], EXP_SHIFT)

        tiles = {}  # per-batch SBUF tiles

        def alloc_proj_tiles(b):
            qT = bigs.tile([F, T], MM_DT, tag="qT", name=f"qT{b}", bufs=3)
            kT = bigs.tile([F, T], MM_DT, tag="kT", name=f"kT{b}", bufs=3)
            # PV stationary, both heads: [keys, ktile, head, hi|ones|lo]
            v1 = bigs.tile([128, NJ, HPC, 128], PV_DT, tag="v1",
                           name=f"v1_{b}", bufs=3)
            nc.vector.tensor_copy(
                v1[:, :, :, D : D + 1],
                ones8[:].rearrange("p (a h b) -> p a h b", h=HPC, b=1),
            )
            tiles[b] = {"qT": qT, "kT": kT, "v1": v1}

        xstage = {}

        def prefetch_x(b):
            """Issue all DMA loads for batch b's activations (no PE work)."""
            t0 = b * T
            for t4 in range(NT4):
                for kk in range(NK_C):
                    xt = xpool.tile([TILE_K, TILE_N], MM_DT, tag="xt",
                                    name=f"xt{b}_{t4}_{kk}")
                    nc.sync.dma_start(
                        xt[:],
                        xT[kk * TILE_K : (kk + 1) * TILE_K,
                           t0 + t4 * TILE_N : t0 + (t4 + 1) * TILE_N],
                    )
                    xstage[b, t4, kk] = xt

        def gen_proj(b):
            """Generator: projections for batch b; yields after each PE op."""
            alloc_proj_tiles(b)
            tl = tiles[b]
            t0 = b * T
            for t4 in range(NT4):
                xts = [xstage.pop((b, t4, kk)) for kk in range(NK_C)]
                for which, w_sb in (("q", wq_sb), ("k", wk_sb), ("v", wv_sb)):
                    acc = ps_aux.tile([128, TILE_N], FP32, tag="aux")
                    for kk in range(NK_C):
                        nc.tensor.matmul(
                            acc[:], w_sb[:, kk * F : (kk + 1) * F], xts[kk][:],
                            start=(kk == 0), stop=(kk == NK_C - 1),
                        )
                        yield
                    if which == "q":
                        nc.vector.tensor_scalar_add(
                            tl["qT"][:, t4 * TILE_N : (t4 + 1) * TILE_N],
                            acc[:], bq_sb[:])
                    elif which == "k":
                        nc.vector.tensor_scalar_add(
                            tl["kT"][:, t4 * TILE_N : (t4 + 1) * TILE_N],
                            acc[:], bk_sb[:])
                    else:
                        vt_sb = vstage.tile([128, TILE_N], MM_DT, tag="vt")
                        nc.vector.tensor_copy(vt_sb[:], acc[:])
                        # transpose [dims, tokens] -> [tokens, dims] on the
                        # DMA XBAR: no PE or PSUM involvement at all
                        vUT = vstage.tile([128, TILE_N // 128, 128], MM_DT,
                                          tag="vUT", bufs=2)
                        for tt in range(TILE_N // 128):
                            nc.sync.dma_start_transpose(
                                vUT[:, tt, :],
                                vt_sb[:, tt * 128 : (tt + 1) * 128])
                        # v1 fp8 hi/lo split, whole 512-token chunk at once.
                        # De-prioritized: v1 is produced a whole batch ahead,
                        # so these must not clog the DVE queue ahead of
                        # release-critical PSUM-draining copies.
                        vh = tl["v1"]
                        jsl4 = slice(t4 * 4, t4 * 4 + 4)
                        v4 = vUT[:].rearrange("p b (h d) -> p b h d", h=HPC)
                        with tc.high_priority(offset=-(1 << 20)):
                            # hi: fp8 round of v
                            nc.vector.tensor_copy(vh[:, jsl4, :, 0:D], v4)
                            # lo: v - hi (exact in fp16), dims 0..62
                            sub = vstage.tile([128, TILE_N // 128, HPC, D],
                                              MM_DT, tag="sub", bufs=2)
                            nc.vector.tensor_sub(
                                sub[:], v4, vh[:, jsl4, :, 0:D])
                            nc.vector.tensor_copy(
                                vh[:, jsl4, :, D + 1 : 128],
                                sub[:, :, :, 0:NLO])

        def gen_wo_t4(b, t4):
            """Generator: output projection chunk; yields per PE op."""
            t0 = b * T
            ctxT = tiles[b]["ctxT"]
            for o in range(C // 128):
                po = ps_aux.tile([128, TILE_N], FP32, tag="aux")
                nc.tensor.matmul(
                    po[:], wo_sb[:, o * 128 : (o + 1) * 128],
                    ctxT[:, t4 * TILE_N : (t4 + 1) * TILE_N],
                    start=True, stop=True,
                )
                osb = vstage.tile([128, TILE_N], OUT_DT, tag="osb", bufs=4)
                nc.vector.tensor_copy(osb[:], po[:])
                nc.sync.dma_start(
                    outT[o * 128 : (o + 1) * 128,
                         t0 + t4 * TILE_N : t0 + (t4 + 1) * TILE_N],
                    osb[:],
                )
                yield

        # persistent lo-correction staging, row 63 stays zero (dim 63 hi-only)
        lo64 = []
        for h in range(HPC):
            lt = consts.tile([D, TILE_N], FP32, name=f"lo64_{h}")
            nc.gpsimd.memset(lt[:], 0.0)
            lo64.append(lt)

        fillers = []

        def pull(budget):
            while budget > 0 and fillers:
                try:
                    next(fillers[0])
                    budget -= 1
                except StopIteration:
                    fillers.pop(0)

        # prologue: projections for batch 0 (PE-only ramp); batch 1's
        # x-loads and projection generator queue up behind it
        prefetch_x(0)
        prefetch_x(1)
        for _ in gen_proj(0):
            pass
        fillers.append(gen_proj(1))

        # ---- flattened software-pipelined attention stream ----
        # one pair-step = 2 key tiles x 2 heads: 2x(QK pair + exp), then the
        # (a few steps earlier, per pv_sched) DoubleRow PV pair.  Chunks of
        # 512 queries flow back-to-back so the Scalar engine never stalls at
        # chunk borders.
        chunks = [(b, i4) for b in range(B) for i4 in range(NT4)]
        S = len(chunks) * NP
        pvs = {}     # chunk idx -> (pv0, pv1)
        expts = {}   # (chunk, pair) -> epair tile

        # pv pair -> emission step: finish each chunk's PV one step into the
        # next chunk so the normalize chain gets ~2 steps before the PSUM
        # banks are rewritten; chunk 0 lags more (v1 is still being built).
        from collections import defaultdict
        pv_sched = defaultdict(list)
        for c in range(len(chunks)):
            offs = [2, 3, 4, 5, 6, 7, 7, 8]
            for p, off in enumerate(offs):
                pv_sched[8 * c + off].append((c, p))
        S_END = max(pv_sched) + 1

        def chunk_start(ci):
            b, i4 = chunks[ci]
            if i4 == 0:
                tiles[b]["ctxT"] = bigs.tile([F, T], MM_DT, tag="ctxT",
                                             name=f"ctxT{b}")
            # queue projections a batch and a half early (triple-buffered
            # qT/kT/v1 make the slot free by then); x DMAs go out first
            if i4 == 2 and b + 2 < B:
                prefetch_x(b + 2)
                fillers.append(gen_proj(b + 2))

        def pv_step(ci, pp):
            b, i4 = chunks[ci]
            v1 = tiles[b]["v1"]
            if pp == 0:
                pvs[ci] = (ps_pv.tile([128, TILE_N], FP32, tag="pv0",
                                      name=f"pv0_{ci}"),
                           ps_pv.tile([128, TILE_N], FP32, tag="pv1",
                                      name=f"pv1_{ci}"))
            e = expts.pop((ci, pp))
            for h, pv in ((0, pvs[ci][0]), (1, pvs[ci][1])):
                nc.tensor.matmul(
                    pv[:], v1[:, 2 * pp : 2 * pp + 2, h, :],
                    e[:, :, h, :], start=(pp == 0), stop=(pp == NP - 1),
                    perf_mode=DR)
            if pp == NP - 1:
                chunk_finish(ci)

        def chunk_finish(ci):
            b, i4 = chunks[ci]
            isl = slice(i4 * TILE_N, (i4 + 1) * TILE_N)
            ctxT = tiles[b]["ctxT"]
            pv0, pv1 = pvs.pop(ci)
            # psum rows: 0-63 ctx_hi, 64 denom, 65-127 lo-correction.
            # One fast copy per head releases the PSUM bank for the next
            # chunk; the whole normalize chain then runs from SBUF.
            Ps = []
            with tc.high_priority(offset=(1 << 19)):
                for h, pv in ((0, pv0), (1, pv1)):
                    P = small.tile([128, TILE_N], FP32, tag=f"P{h}")
                    nc.vector.tensor_copy(P[:], pv[:])
                    Ps.append(P)
            rd = small.tile([1, HPC, TILE_N], FP32, tag="rd")
            dnv = small.tile([1, HPC, TILE_N], FP32, tag="dnv")
            nc.vector.tensor_copy(dnv[:, 0, :], Ps[0][D : D + 1, :])
            nc.vector.tensor_copy(dnv[:, 1, :], Ps[1][D : D + 1, :])
            nc.vector.reciprocal_approx_fast(rd[:], dnv[:])
            bc = small.tile([D, HPC, TILE_N], FP32, tag="bc")
            nc.gpsimd.partition_broadcast(bc[:], rd[:])
            for h, P in ((0, Ps[0]), (1, Ps[1])):
                # shift lo (rows 65..127) down via DMA (engines cannot read
                # unaligned partition bases); lo64 row 63 stays zero
                nc.sync.dma_start(lo64[h][0:NLO, :], P[D + 1 : 128, :])
                hs = small.tile([D, TILE_N], FP32, tag="hs")
                nc.vector.tensor_add(hs[:], P[0:D, :], lo64[h][:])
                nc.vector.tensor_mul(
                    ctxT[h * D : (h + 1) * D, isl], hs[:], bc[:, h, :])
            fillers.append(gen_wo_t4(b, i4))

        started = set()
        for s in range(S_END):
            due = pv_sched.get(s, [])
            if s < S:
                ci, p = divmod(s, NP)
                b, i4 = chunks[ci]
                if p == 0:
                    chunk_start(ci)
                qT, kT = tiles[b]["qT"], tiles[b]["kT"]
                isl = slice(i4 * TILE_N, (i4 + 1) * TILE_N)
                epair = epool.tile([128, 2, HPC, TILE_N], PV_DT, tag="expt")
                expts[ci, p] = epair
                for j2 in range(2):
                    j = 2 * p + j2
                    jsl = slice(j * TILE_K, (j + 1) * TILE_K)
                    qk = ps_qk.tile([128, HPC, TILE_N], FP32, tag="qk")
                    # heads in distinct PE row-groups -> run concurrently
                    nc.tensor.matmul(qk[:, 0, :], kT[0:D, jsl],
                                     qT[0:D, isl], start=True, stop=True)
                    nc.tensor.matmul(qk[:, 1, :],
                                     kT[D : 2 * D, jsl], qT[D : 2 * D, isl],
                                     start=True, stop=True)
                    nc.scalar.activation(epair[:, j2], qk[:], Exp,
                                         bias=nln16[:])
                    # keep the in-order PE stream fed while ACT runs exp
                    if j2 == 0:
                        for c2, p2 in due[0:1]:
                            pv_step(c2, p2)
                    else:
                        for c2, p2 in due[1:]:
                            pv_step(c2, p2)
                    pull(2 if j2 == 0 else 3)
            else:
                for c2, p2 in due:
                    pv_step(c2, p2)

        # drain remaining fillers (last batch's final wo chunks)
        pull(10 ** 9)


_CACHE = {}


def _get_nc():
    if "nc" not in _CACHE:
        nc = bacc.Bacc("TRN2", target_bir_lowering=False, debug=False,
                       num_devices=NCORES)
        with tile.TileContext(nc) as tc:
            build_kernel_body(tc)
        nc.compile()
        _CACHE["nc"] = nc
    return _CACHE["nc"]


def host_prep(x, Wq, bq, Wk, bk, Wv, bv, Wo, bo):
    f16 = np.float16
    x = np.asarray(x, np.float32)
    xT = np.ascontiguousarray(x.reshape(TT, C).T.astype(f16))
    scale = np.float32(1.0 / np.sqrt(D))
    in_maps = []
    for c in range(NCORES):
        fsl = slice(c * F, (c + 1) * F)
        in_maps.append({
            "xT": xT,
            "wq": np.ascontiguousarray(
                (np.asarray(Wq, np.float32)[:, fsl] * scale).astype(f16)),
            "wk": np.ascontiguousarray(np.asarray(Wk, np.float32)[:, fsl].astype(f16)),
            "wv": np.ascontiguousarray(np.asarray(Wv, np.float32)[:, fsl].astype(f16)),
            "wo": np.ascontiguousarray(np.asarray(Wo, np.float32)[fsl, :].astype(f16)),
            "bq": np.ascontiguousarray(np.asarray(bq, np.float32)[fsl] * scale),
            "bk": np.ascontiguousarray(np.asarray(bk, np.float32)[fsl]),
        })
    return in_maps


def host_gather(results, Wo, bo, bv):
    total = np.zeros((C, TT), np.float32)
    for c in range(NCORES):
        total += results[c]["outT"].astype(np.float32)
    out = total.T
    out = out + (np.asarray(bo, np.float32)
                 + np.asarray(bv, np.float32) @ np.asarray(Wo, np.float32))
    return out.reshape(B, T, C)


def _install_profile_hook():
    """Make trace=True work under axon when antenv.axon_hooks is absent."""
    import sys
    import types

    try:
        import antenv.axon_hooks  # noqa: F401
        return
    except ImportError:
        pass
    import antenv
    from trn_agent_boot.trn_boot import _ntff_profile_via_ctypes

    mod = types.ModuleType("antenv.axon_hooks")
    holder = [None]
    mod.set_axon_ntff_profile_hook = lambda h: holder.__setitem__(0, h)
    mod.get_axon_ntff_profile_hook = lambda: holder[0]
    sys.modules["antenv.axon_hooks"] = mod
    antenv.axon_hooks = mod
    mod.set_axon_ntff_profile_hook(
        _ntff_profile_via_ctypes("/opt/axon/libaxon_pjrt.so")
    )
    # artifact upload needs internal storage; keep profiles local
    import concourse.bass_utils as bu
    bu.upload_artifacts = lambda tmpdir: f"local:{tmpdir}"


def kernel(x, Wq, bq, Wk, bk, Wv, bv, Wo, bo, _trace=False):
    if _trace:
        _install_profile_hook()
    nc = _get_nc()
    in_maps = host_prep(x, Wq, bq, Wk, bk, Wv, bv, Wo, bo)
    res = run_bass_kernel_spmd(nc, in_maps, core_ids=list(range(NCORES)),
                               trace=_trace)
    _CACHE["last_result"] = res
    return host_gather(res.results, Wo, bo, bv)


# revision 40
# speedup vs baseline: 1.0592x; 1.0592x over previous
"""Multi-head self-attention (B=4, T=2048, C=1024, H=16, D=64) on 8 NeuronCores.

Sharding: tensor-parallel over heads (Megatron): each core owns 2 heads.
Wq/Wk/Wv column-sharded, Wo row-sharded; host sums the 8 partial outputs.

Device layout is fully "transposed" (features on partitions, tokens on the
free dim) so that softmax runs over the PSUM free dim and the PV matmul needs
no attention-matrix transpose.

Precision plan: fp16 everywhere except the PV contraction, which runs in
fp8e4 DoubleRow mode (contracting two 128-key tiles per matmul => half the
PE streaming time). To keep accuracy, V is split hi/lo: the 128 stationary
columns of the PV matmul are [v_hi(64) | ones(1) | v_lo(63)], so the PSUM
holds ctx_hi (rows 0-63), the softmax denominator (row 64), and an fp8
error-correction term (rows 65-127) that is added back on the DVE.
exp() output is written as fp8e4 directly by the Scalar engine with a
-ln(16) bias shift to keep values inside the e4m3 range (cancels in the
softmax normalization).

The PE executes its stream in order, so projection/output-projection matmuls
of adjacent batches are interleaved into the attention j-loop (which is
paced by the Scalar engine's exp) to keep both engines near 100% busy.
"""

import numpy as np

import concourse.bass as bass
import concourse.tile as tile
from concourse import bacc, mybir
from concourse.bass_utils import run_bass_kernel_spmd

B, T, C, H, D = 4, 2048, 1024, 16, 64
NCORES = 8
HPC = H // NCORES          # heads per core = 2
F = HPC * D                # per-core feature width = 128
TT = B * T                 # total tokens = 8192

FP32 = mybir.dt.float32
FP16 = mybir.dt.float16
MM_DT = mybir.dt.float16   # matmul compute dtype
PV_DT = mybir.dt.float8e4  # PV matmul dtype (DoubleRow: 2 key-tiles per MM)
OUT_DT = mybir.dt.float16  # partial-output DMA dtype

TILE_K = 128               # contraction tile
TILE_N = 512               # moving free dim per matmul
NK_C = C // TILE_K         # 8 k-tiles over channels
NT4 = T // TILE_N          # 4 token chunks per batch
NJ = T // TILE_K           # 16 key tiles per batch
NP = NJ // 2               # 8 key-tile PAIRS per batch (DoubleRow)
NLO = D - 1                # 63 lo-correction dims (dim 63 is hi-only)

EXP_SHIFT = -2.772588722239781   # -ln(16): keep exp() inside e4m3 range


def build_kernel_body(tc):
    nc = tc.nc
    Exp = mybir.ActivationFunctionType.Exp
    DR = mybir.MatmulPerfMode.DoubleRow

    xT = nc.dram_tensor("xT", [C, TT], MM_DT, kind="ExternalInput").ap()
    wq = nc.dram_tensor("wq", [C, F], MM_DT, kind="ExternalInput").ap()
    wk = nc.dram_tensor("wk", [C, F], MM_DT, kind="ExternalInput").ap()
    wv = nc.dram_tensor("wv", [C, F], MM_DT, kind="ExternalInput").ap()
    wo = nc.dram_tensor("wo", [F, C], MM_DT, kind="ExternalInput").ap()
    bqv = nc.dram_tensor("bq", [F], FP32, kind="ExternalInput").ap()
    bkv = nc.dram_tensor("bk", [F], FP32, kind="ExternalInput").ap()
    outT = nc.dram_tensor("outT", [C, TT], OUT_DT, kind="ExternalOutput").ap()

    import contextlib
    ctx = contextlib.ExitStack()
    with ctx:
        consts = ctx.enter_context(tc.tile_pool(name="consts", bufs=1))
        xpool = ctx.enter_context(tc.tile_pool(name="xt", bufs=64))
        bigs = ctx.enter_context(tc.tile_pool(name="bigs", bufs=2))
        epool = ctx.enter_context(tc.tile_pool(name="expp", bufs=6))
        small = ctx.enter_context(tc.tile_pool(name="small", bufs=2))
        vstage = ctx.enter_context(tc.tile_pool(name="vstage", bufs=2))
        ps_qk = ctx.enter_context(tc.tile_pool(name="ps_qk", bufs=2, space="PSUM"))
        ps_pv = ctx.enter_context(tc.tile_pool(name="ps_pv", bufs=1, space="PSUM"))
        ps_aux = ctx.enter_context(tc.tile_pool(name="ps_aux", bufs=2, space="PSUM"))

        # ---- constants ----
        wq_sb = consts.tile([TILE_K, C], MM_DT)  # c-tile k at [:, k*F:(k+1)*F]
        nc.sync.dma_start(
            wq_sb[:].rearrange("p (k f) -> p k f", k=NK_C),
            wq.rearrange("(k p) f -> p k f", p=TILE_K))
        wk_sb = consts.tile([TILE_K, C], MM_DT)
        nc.sync.dma_start(
            wk_sb[:].rearrange("p (k f) -> p k f", k=NK_C),
            wk.rearrange("(k p) f -> p k f", p=TILE_K))
        wv_sb = consts.tile([TILE_K, C], MM_DT)
        nc.sync.dma_start(
            wv_sb[:].rearrange("p (k f) -> p k f", p=TILE_K, k=NK_C),
            wv.rearrange("(k p) f -> p k f", p=TILE_K))
        wo_sb = consts.tile([F, C], MM_DT)
        nc.sync.dma_start(wo_sb[:], wo)
        bq_sb = consts.tile([F, 1], FP32)
        nc.sync.dma_start(bq_sb[:], bqv.rearrange("(p one) -> p one", one=1))
        bk_sb = consts.tile([F, 1], FP32)
        nc.sync.dma_start(bk_sb[:], bkv.rearrange("(p one) -> p one", one=1))
        ident32 = consts.tile([128, 128], FP32)
        from concourse.masks import make_identity
        make_identity(nc, ident32[:])
        ident = consts.tile([128, 128], MM_DT)
        nc.vector.tensor_copy(ident[:], ident32[:])
        ones8 = consts.tile([128, NJ * HPC], PV_DT)
        nc.gpsimd.memset(ones8[:], 1.0)
        nln16 = consts.tile([128, 1], FP32)
        nc.gpsimd.memset(nln16[:], EXP_SHIFT)

        tiles = {}  # per-batch SBUF tiles

        def alloc_proj_tiles(b):
            qT = bigs.tile([F, T], MM_DT, tag="qT", name=f"qT{b}", bufs=3)
            kT = bigs.tile([F, T], MM_DT, tag="kT", name=f"kT{b}", bufs=3)
            # PV stationary, both heads: [keys, ktile, head, hi|ones|lo]
            v1 = bigs.tile([128, NJ, HPC, 128], PV_DT, tag="v1",
                           name=f"v1_{b}", bufs=3)
            nc.vector.tensor_copy(
                v1[:, :, :, D : D + 1],
                ones8[:].rearrange("p (a h b) -> p a h b", h=HPC, b=1),
            )
            tiles[b] = {"qT": qT, "kT": kT, "v1": v1}

        xstage = {}

        def gen_xt(b):
            """Generator: issue batch b's x DMA loads, spread out so the
            Sync queue is not clogged by a 32-DMA burst."""
            t0 = b * T
            for t4 in range(NT4):
                for kk in range(NK_C):
                    xt = xpool.tile([TILE_K, TILE_N], MM_DT, tag="xt",
                                    name=f"xt{b}_{t4}_{kk}")
                    nc.sync.dma_start(
                        xt[:],
                        xT[kk * TILE_K : (kk + 1) * TILE_K,
                           t0 + t4 * TILE_N : t0 + (t4 + 1) * TILE_N],
                    )
                    xstage[b, t4, kk] = xt
                    if kk % 4 == 3:
                        yield

        def prefetch_x(b):
            for _ in gen_xt(b):
                pass

        def gen_proj(b):
            """Generator: projections for batch b; yields after each PE op."""
            alloc_proj_tiles(b)
            tl = tiles[b]
            t0 = b * T
            for t4 in range(NT4):
                xts = [xstage.pop((b, t4, kk)) for kk in range(NK_C)]
                for which, w_sb in (("q", wq_sb), ("k", wk_sb), ("v", wv_sb)):
                    acc = ps_aux.tile([128, TILE_N], FP32, tag="aux")
                    for kk in range(NK_C):
                        nc.tensor.matmul(
                            acc[:], w_sb[:, kk * F : (kk + 1) * F], xts[kk][:],
                            start=(kk == 0), stop=(kk == NK_C - 1),
                        )
                        yield
                    if which == "q":
                        nc.vector.tensor_scalar_add(
                            tl["qT"][:, t4 * TILE_N : (t4 + 1) * TILE_N],
                            acc[:], bq_sb[:])
                    elif which == "k":
                        nc.vector.tensor_scalar_add(
                            tl["kT"][:, t4 * TILE_N : (t4 + 1) * TILE_N],
                            acc[:], bk_sb[:])
                    else:
                        vt_sb = vstage.tile([128, TILE_N], MM_DT, tag="vt")
                        nc.vector.tensor_copy(vt_sb[:], acc[:])
                        # transpose [dims, tokens] -> [tokens, dims] on the
                        # PE; all four 128-blocks share one PSUM slot, then
                        # one fast high-priority copy releases it
                        ptr4 = ps_aux.tile([128, TILE_N], MM_DT, tag="aux")
                        for tt in range(TILE_N // 128):
                            nc.tensor.transpose(
                                ptr4[:, tt * 128 : (tt + 1) * 128],
                                vt_sb[:, tt * 128 : (tt + 1) * 128], ident[:])
                            yield
                        vUT = vstage.tile([128, TILE_N // 128, 128], MM_DT,
                                          tag="vUT", bufs=2)
                        with tc.high_priority(offset=(1 << 19)):
                            nc.vector.tensor_copy(
                                vUT[:].rearrange("p b d -> p (b d)"), ptr4[:])
                        # v1 fp8 hi/lo split, whole 512-token chunk at once.
                        # De-prioritized: v1 is produced a whole batch ahead,
                        # so these must not clog the DVE queue ahead of
                        # release-critical PSUM-draining copies.
                        vh = tl["v1"]
                        jsl4 = slice(t4 * 4, t4 * 4 + 4)
                        v4 = vUT[:].rearrange("p b (h d) -> p b h d", h=HPC)
                        with tc.high_priority(offset=-(1 << 20)):
                            # hi: fp8 round of v
                            nc.vector.tensor_copy(vh[:, jsl4, :, 0:D], v4)
                            # lo: v - hi (exact in fp16), dims 0..62
                            sub = vstage.tile([128, TILE_N // 128, HPC, D],
                                              MM_DT, tag="sub", bufs=2)
                            nc.vector.tensor_sub(
                                sub[:], v4, vh[:, jsl4, :, 0:D])
                            nc.vector.tensor_copy(
                                vh[:, jsl4, :, D + 1 : 128],
                                sub[:, :, :, 0:NLO])

        def gen_wo_t4(b, t4):
            """Generator: output projection chunk; yields per PE op."""
            t0 = b * T
            ctxT = tiles[b]["ctxT"]
            for o in range(C // 128):
                po = ps_aux.tile([128, TILE_N], FP32, tag="aux")
                nc.tensor.matmul(
                    po[:], wo_sb[:, o * 128 : (o + 1) * 128],
                    ctxT[:, t4 * TILE_N : (t4 + 1) * TILE_N],
                    start=True, stop=True,
                )
                osb = vstage.tile([128, TILE_N], OUT_DT, tag="osb", bufs=4)
                nc.vector.tensor_copy(osb[:], po[:])
                nc.sync.dma_start(
                    outT[o * 128 : (o + 1) * 128,
                         t0 + t4 * TILE_N : t0 + (t4 + 1) * TILE_N],
                    osb[:],
                )
                yield

        # persistent lo-correction staging, row 63 stays zero (dim 63 hi-only)
        lo64 = []
        for h in range(HPC):
            lt = consts.tile([D, TILE_N], FP32, name=f"lo64_{h}")
            nc.gpsimd.memset(lt[:], 0.0)
            lo64.append(lt)

        fillers = []

        def pull(budget):
            while budget > 0 and fillers:
                try:
                    next(fillers[0])
                    budget -= 1
                except StopIteration:
                    fillers.pop(0)

        # prologue: projections for batch 0 (PE-only ramp); batch 1's
        # x-loads and projection generator queue up behind it
        prefetch_x(0)
        prefetch_x(1)
        for _ in gen_proj(0):
            pass
        fillers.append(gen_proj(1))

        # ---- flattened software-pipelined attention stream ----
        # one pair-step = 2 key tiles x 2 heads: 2x(QK pair + exp), then the
        # (a few steps earlier, per pv_sched) DoubleRow PV pair.  Chunks of
        # 512 queries flow back-to-back so the Scalar engine never stalls at
        # chunk borders.
        chunks = [(b, i4) for b in range(B) for i4 in range(NT4)]
        S = len(chunks) * NP
        pvs = {}     # chunk idx -> (pv0, pv1)
        expts = {}   # (chunk, pair) -> epair tile

        # pv pair -> emission step: finish each chunk's PV one step into the
        # next chunk so the normalize chain gets ~2 steps before the PSUM
        # banks are rewritten; chunk 0 lags more (v1 is still being built).
        from collections import defaultdict
        pv_sched = defaultdict(list)
        for c in range(len(chunks)):
            offs = [2, 3, 4, 5, 6, 7, 7, 8]
            for p, off in enumerate(offs):
                pv_sched[8 * c + off].append((c, p))
        S_END = max(pv_sched) + 1

        def chunk_start(ci):
            b, i4 = chunks[ci]
            if i4 == 0:
                tiles[b]["ctxT"] = bigs.tile([F, T], MM_DT, tag="ctxT",
                                             name=f"ctxT{b}")
            # queue projections a batch and a half early (triple-buffered
            # qT/kT/v1 make the slot free by then); x DMAs go out first,
            # spread across pulls
            if i4 == 1 and b + 2 < B:
                fillers.append(gen_xt(b + 2))
            if i4 == 2 and b + 2 < B:
                fillers.append(gen_proj(b + 2))

        def pv_step(ci, pp):
            b, i4 = chunks[ci]
            v1 = tiles[b]["v1"]
            if pp == 0:
                pvs[ci] = (ps_pv.tile([128, TILE_N], FP32, tag="pv0",
                                      name=f"pv0_{ci}"),
                           ps_pv.tile([128, TILE_N], FP32, tag="pv1",
                                      name=f"pv1_{ci}"))
            e = expts.pop((ci, pp))
            for h, pv in ((0, pvs[ci][0]), (1, pvs[ci][1])):
                nc.tensor.matmul(
                    pv[:], v1[:, 2 * pp : 2 * pp + 2, h, :],
                    e[:, :, h, :], start=(pp == 0), stop=(pp == NP - 1),
                    perf_mode=DR)
            if pp == NP - 1:
                chunk_finish(ci)

        def chunk_finish(ci):
            b, i4 = chunks[ci]
            isl = slice(i4 * TILE_N, (i4 + 1) * TILE_N)
            ctxT = tiles[b]["ctxT"]
            pv0, pv1 = pvs.pop(ci)
            # psum rows: 0-63 ctx_hi, 64 denom, 65-127 lo-correction.
            # One fast copy per head releases the PSUM bank for the next
            # chunk; the whole normalize chain then runs from SBUF.
            Ps = []
            with tc.high_priority(offset=(1 << 19)):
                for h, pv in ((0, pv0), (1, pv1)):
                    P = small.tile([128, TILE_N], FP32, tag=f"P{h}")
                    nc.vector.tensor_copy(P[:], pv[:])
                    Ps.append(P)
            rd = small.tile([1, HPC, TILE_N], FP32, tag="rd")
            dnv = small.tile([1, HPC, TILE_N], FP32, tag="dnv")
            nc.vector.tensor_copy(dnv[:, 0, :], Ps[0][D : D + 1, :])
            nc.vector.tensor_copy(dnv[:, 1, :], Ps[1][D : D + 1, :])
            nc.vector.reciprocal_approx_fast(rd[:], dnv[:])
            bc = small.tile([D, HPC, TILE_N], FP32, tag="bc")
            nc.gpsimd.partition_broadcast(bc[:], rd[:])
            for h, P in ((0, Ps[0]), (1, Ps[1])):
                # shift lo (rows 65..127) down one partition via DMA
                # (engines cannot read unaligned partition bases);
                # lo64 row 63 stays zero (dim 63 is hi-only)
                nc.sync.dma_start(lo64[h][0:NLO, :], P[D + 1 : 128, :])
                hs = small.tile([D, TILE_N], FP32, tag="hs")
                nc.vector.tensor_add(hs[:], P[0:D, :], lo64[h][:])
                nc.vector.tensor_mul(
                    ctxT[h * D : (h + 1) * D, isl], hs[:], bc[:, h, :])
            fillers.append(gen_wo_t4(b, i4))

        started = set()
        for s in range(S_END):
            due = pv_sched.get(s, [])
            if s < S:
                ci, p = divmod(s, NP)
                b, i4 = chunks[ci]
                if p == 0:
                    chunk_start(ci)
                qT, kT = tiles[b]["qT"], tiles[b]["kT"]
                isl = slice(i4 * TILE_N, (i4 + 1) * TILE_N)
                epair = epool.tile([128, 2, HPC, TILE_N], PV_DT, tag="expt")
                expts[ci, p] = epair
                for j2 in range(2):
                    j = 2 * p + j2
                    jsl = slice(j * TILE_K, (j + 1) * TILE_K)
                    qk = ps_qk.tile([128, HPC, TILE_N], FP32, tag="qk")
                    # heads in distinct PE row-groups -> run concurrently
                    nc.tensor.matmul(qk[:, 0, :], kT[0:D, jsl],
                                     qT[0:D, isl], start=True, stop=True)
                    nc.tensor.matmul(qk[:, 1, :],
                                     kT[D : 2 * D, jsl], qT[D : 2 * D, isl],
                                     start=True, stop=True)
                    nc.scalar.activation(epair[:, j2], qk[:], Exp,
                                         bias=nln16[:])
                    # keep the in-order PE stream fed while ACT runs exp
                    if j2 == 0:
                        for c2, p2 in due[0:1]:
                            pv_step(c2, p2)
                    else:
                        for c2, p2 in due[1:]:
                            pv_step(c2, p2)
                    pull(2 if j2 == 0 else 3)
            else:
                for c2, p2 in due:
                    pv_step(c2, p2)

        # drain remaining fillers (last batch's final wo chunks)
        pull(10 ** 9)


_CACHE = {}


def _get_nc():
    if "nc" not in _CACHE:
        nc = bacc.Bacc("TRN2", target_bir_lowering=False, debug=False,
                       num_devices=NCORES)
        with tile.TileContext(nc) as tc:
            build_kernel_body(tc)
        nc.compile()
        _CACHE["nc"] = nc
    return _CACHE["nc"]


def host_prep(x, Wq, bq, Wk, bk, Wv, bv, Wo, bo):
    f16 = np.float16
    x = np.asarray(x, np.float32)
    xT = np.ascontiguousarray(x.reshape(TT, C).T.astype(f16))
    scale = np.float32(1.0 / np.sqrt(D))
    in_maps = []
    for c in range(NCORES):
        fsl = slice(c * F, (c + 1) * F)
        in_maps.append({
            "xT": xT,
            "wq": np.ascontiguousarray(
                (np.asarray(Wq, np.float32)[:, fsl] * scale).astype(f16)),
            "wk": np.ascontiguousarray(np.asarray(Wk, np.float32)[:, fsl].astype(f16)),
            "wv": np.ascontiguousarray(np.asarray(Wv, np.float32)[:, fsl].astype(f16)),
            "wo": np.ascontiguousarray(np.asarray(Wo, np.float32)[fsl, :].astype(f16)),
            "bq": np.ascontiguousarray(np.asarray(bq, np.float32)[fsl] * scale),
            "bk": np.ascontiguousarray(np.asarray(bk, np.float32)[fsl]),
        })
    return in_maps


def host_gather(results, Wo, bo, bv):
    total = np.zeros((C, TT), np.float32)
    for c in range(NCORES):
        total += results[c]["outT"].astype(np.float32)
    out = total.T
    out = out + (np.asarray(bo, np.float32)
                 + np.asarray(bv, np.float32) @ np.asarray(Wo, np.float32))
    return out.reshape(B, T, C)


def _install_profile_hook():
    """Make trace=True work under axon when antenv.axon_hooks is absent."""
    import sys
    import types

    try:
        import antenv.axon_hooks  # noqa: F401
        return
    except ImportError:
        pass
    import antenv
    from trn_agent_boot.trn_boot import _ntff_profile_via_ctypes

    mod = types.ModuleType("antenv.axon_hooks")
    holder = [None]
    mod.set_axon_ntff_profile_hook = lambda h: holder.__setitem__(0, h)
    mod.get_axon_ntff_profile_hook = lambda: holder[0]
    sys.modules["antenv.axon_hooks"] = mod
    antenv.axon_hooks = mod
    mod.set_axon_ntff_profile_hook(
        _ntff_profile_via_ctypes("/opt/axon/libaxon_pjrt.so")
    )
    # artifact upload needs internal storage; keep profiles local
    import concourse.bass_utils as bu
    bu.upload_artifacts = lambda tmpdir: f"local:{tmpdir}"


def kernel(x, Wq, bq, Wk, bk, Wv, bv, Wo, bo, _trace=False):
    if _trace:
        _install_profile_hook()
    nc = _get_nc()
    in_maps = host_prep(x, Wq, bq, Wk, bk, Wv, bv, Wo, bo)
    res = run_bass_kernel_spmd(nc, in_maps, core_ids=list(range(NCORES)),
                               trace=_trace)
    _CACHE["last_result"] = res
    return host_gather(res.results, Wo, bo, bv)


# revision 42
# speedup vs baseline: 1.2600x; 1.1896x over previous
"""Multi-head self-attention (B=4, T=2048, C=1024, H=16, D=64) on 8 NeuronCores.

Sharding: tensor-parallel over heads (Megatron): each core owns 2 heads.
Wq/Wk/Wv column-sharded, Wo row-sharded; host sums the 8 partial outputs.

Device layout is fully "transposed" (features on partitions, tokens on the
free dim) so that softmax runs over the PSUM free dim and the PV matmul needs
no attention-matrix transpose.

Precision plan: fp16 everywhere except the PV contraction, which runs in
fp8e4 DoubleRow mode (contracting two 128-key tiles per matmul => half the
PE streaming time). To keep accuracy, V is split hi/lo: the 128 stationary
columns of the PV matmul are [v_hi(64) | ones(1) | v_lo(63)], so the PSUM
holds ctx_hi (rows 0-63), the softmax denominator (row 64), and an fp8
error-correction term (rows 65-127) that is added back on the DVE.
exp() output is written as fp8e4 directly by the Scalar engine with a
-ln(16) bias shift to keep values inside the e4m3 range (cancels in the
softmax normalization).

The PE executes its stream in order, so projection/output-projection matmuls
of adjacent batches are interleaved into the attention j-loop (which is
paced by the Scalar engine's exp) to keep both engines near 100% busy.
"""

import numpy as np

import concourse.bass as bass
import concourse.tile as tile
from concourse import bacc, mybir
from concourse.bass_utils import run_bass_kernel_spmd

B, T, C, H, D = 4, 2048, 1024, 16, 64
NCORES = 8
HPC = H // NCORES          # heads per core = 2
F = HPC * D                # per-core feature width = 128
TT = B * T                 # total tokens = 8192

FP32 = mybir.dt.float32
FP16 = mybir.dt.float16
MM_DT = mybir.dt.float16   # matmul compute dtype
PV_DT = mybir.dt.float8e4  # PV matmul dtype (DoubleRow: 2 key-tiles per MM)
OUT_DT = mybir.dt.float16  # partial-output DMA dtype

TILE_K = 128               # contraction tile
TILE_N = 512               # moving free dim per matmul
NK_C = C // TILE_K         # 8 k-tiles over channels
NT4 = T // TILE_N          # 4 token chunks per batch
NJ = T // TILE_K           # 16 key tiles per batch
NP = NJ // 2               # 8 key-tile PAIRS per batch (DoubleRow)
NLO = D - 1                # 63 lo-correction dims (dim 63 is hi-only)

EXP_SHIFT = -2.772588722239781   # -ln(16): keep exp() inside e4m3 range


def build_kernel_body(tc):
    nc = tc.nc
    Exp = mybir.ActivationFunctionType.Exp
    DR = mybir.MatmulPerfMode.DoubleRow

    xT = nc.dram_tensor("xT", [C, TT], MM_DT, kind="ExternalInput").ap()
    wq = nc.dram_tensor("wq", [C, F], MM_DT, kind="ExternalInput").ap()
    wk = nc.dram_tensor("wk", [C, F], MM_DT, kind="ExternalInput").ap()
    wv = nc.dram_tensor("wv", [C, F], MM_DT, kind="ExternalInput").ap()
    wo = nc.dram_tensor("wo", [F, C], MM_DT, kind="ExternalInput").ap()
    bqv = nc.dram_tensor("bq", [F], FP32, kind="ExternalInput").ap()
    bkv = nc.dram_tensor("bk", [F], FP32, kind="ExternalInput").ap()
    outT = nc.dram_tensor("outT", [C, TT], OUT_DT, kind="ExternalOutput").ap()

    import contextlib
    ctx = contextlib.ExitStack()
    with ctx:
        consts = ctx.enter_context(tc.tile_pool(name="consts", bufs=1))
        xpool = ctx.enter_context(tc.tile_pool(name="xt", bufs=64))
        bigs = ctx.enter_context(tc.tile_pool(name="bigs", bufs=2))
        epool = ctx.enter_context(tc.tile_pool(name="expp", bufs=6))
        small = ctx.enter_context(tc.tile_pool(name="small", bufs=2))
        vstage = ctx.enter_context(tc.tile_pool(name="vstage", bufs=2))
        ps_qk = ctx.enter_context(tc.tile_pool(name="ps_qk", bufs=2, space="PSUM"))
        ps_pv = ctx.enter_context(tc.tile_pool(name="ps_pv", bufs=1, space="PSUM"))
        ps_aux = ctx.enter_context(tc.tile_pool(name="ps_aux", bufs=2, space="PSUM"))

        # ---- constants ----
        wq_sb = consts.tile([TILE_K, C], MM_DT)  # c-tile k at [:, k*F:(k+1)*F]
        nc.sync.dma_start(
            wq_sb[:].rearrange("p (k f) -> p k f", k=NK_C),
            wq.rearrange("(k p) f -> p k f", p=TILE_K))
        wk_sb = consts.tile([TILE_K, C], MM_DT)
        nc.sync.dma_start(
            wk_sb[:].rearrange("p (k f) -> p k f", k=NK_C),
            wk.rearrange("(k p) f -> p k f", p=TILE_K))
        wv_sb = consts.tile([TILE_K, C], MM_DT)
        nc.sync.dma_start(
            wv_sb[:].rearrange("p (k f) -> p k f", p=TILE_K, k=NK_C),
            wv.rearrange("(k p) f -> p k f", p=TILE_K))
        wo_sb = consts.tile([F, C], MM_DT)
        nc.sync.dma_start(wo_sb[:], wo)
        bq_sb = consts.tile([F, 1], FP32)
        nc.sync.dma_start(bq_sb[:], bqv.rearrange("(p one) -> p one", one=1))
        bk_sb = consts.tile([F, 1], FP32)
        nc.sync.dma_start(bk_sb[:], bkv.rearrange("(p one) -> p one", one=1))
        ident32 = consts.tile([128, 128], FP32)
        from concourse.masks import make_identity
        make_identity(nc, ident32[:])
        ident = consts.tile([128, 128], MM_DT)
        nc.vector.tensor_copy(ident[:], ident32[:])
        ones8 = consts.tile([128, NJ * HPC], PV_DT)
        nc.gpsimd.memset(ones8[:], 1.0)
        nln16 = consts.tile([128, 1], FP32)
        nc.gpsimd.memset(nln16[:], EXP_SHIFT)

        tiles = {}  # per-batch SBUF tiles

        def alloc_proj_tiles(b):
            qT = bigs.tile([F, T], MM_DT, tag="qT", name=f"qT{b}")
            kT = bigs.tile([F, T], MM_DT, tag="kT", name=f"kT{b}")
            # PV stationary, both heads: [keys, ktile, head, hi|ones|lo]
            v1 = bigs.tile([128, NJ, HPC, 128], PV_DT, tag="v1",
                           name=f"v1_{b}")
            nc.vector.tensor_copy(
                v1[:, :, :, D : D + 1],
                ones8[:].rearrange("p (a h b) -> p a h b", h=HPC, b=1),
            )
            tiles[b] = {"qT": qT, "kT": kT, "v1": v1}

        xstage = {}

        def gen_xt(b):
            """Generator: issue batch b's x DMA loads, spread out so the
            Sync queue is not clogged by a 32-DMA burst."""
            t0 = b * T
            for t4 in range(NT4):
                for kk in range(NK_C):
                    xt = xpool.tile([TILE_K, TILE_N], MM_DT, tag="xt",
                                    name=f"xt{b}_{t4}_{kk}")
                    nc.sync.dma_start(
                        xt[:],
                        xT[kk * TILE_K : (kk + 1) * TILE_K,
                           t0 + t4 * TILE_N : t0 + (t4 + 1) * TILE_N],
                    )
                    xstage[b, t4, kk] = xt
                    if kk % 4 == 3:
                        yield

        def prefetch_x(b):
            for _ in gen_xt(b):
                pass

        def gen_proj(b):
            """Generator: projections for batch b; yields after each PE op."""
            alloc_proj_tiles(b)
            tl = tiles[b]
            t0 = b * T
            for t4 in range(NT4):
                xts = [xstage.pop((b, t4, kk)) for kk in range(NK_C)]
                for which, w_sb in (("q", wq_sb), ("k", wk_sb), ("v", wv_sb)):
                    acc = ps_aux.tile([128, TILE_N], FP32, tag="aux")
                    for kk in range(NK_C):
                        nc.tensor.matmul(
                            acc[:], w_sb[:, kk * F : (kk + 1) * F], xts[kk][:],
                            start=(kk == 0), stop=(kk == NK_C - 1),
                        )
                        yield
                    if which == "q":
                        nc.vector.tensor_scalar_add(
                            tl["qT"][:, t4 * TILE_N : (t4 + 1) * TILE_N],
                            acc[:], bq_sb[:])
                    elif which == "k":
                        nc.vector.tensor_scalar_add(
                            tl["kT"][:, t4 * TILE_N : (t4 + 1) * TILE_N],
                            acc[:], bk_sb[:])
                    else:
                        vt_sb = vstage.tile([128, TILE_N], MM_DT, tag="vt")
                        nc.vector.tensor_copy(vt_sb[:], acc[:])
                        # transpose [dims, tokens] -> [tokens, dims] on the
                        # PE; all four 128-blocks share one PSUM slot, then
                        # one fast high-priority copy releases it
                        ptr4 = ps_aux.tile([128, TILE_N], MM_DT, tag="aux")
                        for tt in range(TILE_N // 128):
                            nc.tensor.transpose(
                                ptr4[:, tt * 128 : (tt + 1) * 128],
                                vt_sb[:, tt * 128 : (tt + 1) * 128], ident[:])
                            yield
                        vUT = vstage.tile([128, TILE_N // 128, 128], MM_DT,
                                          tag="vUT", bufs=2)
                        nc.vector.tensor_copy(
                            vUT[:].rearrange("p b d -> p (b d)"), ptr4[:])
                        # v1 fp8 hi/lo split, whole 512-token chunk at once.
                        # De-prioritized: v1 is produced a whole batch ahead,
                        # so these must not clog the DVE queue ahead of
                        # release-critical PSUM-draining copies.
                        vh = tl["v1"]
                        jsl4 = slice(t4 * 4, t4 * 4 + 4)
                        v4 = vUT[:].rearrange("p b (h d) -> p b h d", h=HPC)
                        # hi: fp8 round of v
                        nc.vector.tensor_copy(vh[:, jsl4, :, 0:D], v4)
                        # lo: v - hi (exact in fp16), dims 0..62
                        sub = vstage.tile([128, TILE_N // 128, HPC, D],
                                          MM_DT, tag="sub", bufs=2)
                        nc.vector.tensor_sub(
                            sub[:], v4, vh[:, jsl4, :, 0:D])
                        nc.vector.tensor_copy(
                            vh[:, jsl4, :, D + 1 : 128],
                            sub[:, :, :, 0:NLO])

        def gen_wo_t4(b, t4):
            """Generator: output projection chunk; yields per PE op."""
            t0 = b * T
            ctxT = tiles[b]["ctxT"]
            for o in range(C // 128):
                po = ps_aux.tile([128, TILE_N], FP32, tag="aux")
                nc.tensor.matmul(
                    po[:], wo_sb[:, o * 128 : (o + 1) * 128],
                    ctxT[:, t4 * TILE_N : (t4 + 1) * TILE_N],
                    start=True, stop=True,
                )
                osb = vstage.tile([128, TILE_N], OUT_DT, tag="osb", bufs=4)
                nc.vector.tensor_copy(osb[:], po[:])
                nc.sync.dma_start(
                    outT[o * 128 : (o + 1) * 128,
                         t0 + t4 * TILE_N : t0 + (t4 + 1) * TILE_N],
                    osb[:],
                )
                yield

        # persistent lo-correction staging, row 63 stays zero (dim 63 hi-only)
        lo64 = []
        for h in range(HPC):
            lt = consts.tile([D, TILE_N], FP32, name=f"lo64_{h}")
            nc.gpsimd.memset(lt[:], 0.0)
            lo64.append(lt)

        fillers = []

        def pull(budget):
            while budget > 0 and fillers:
                try:
                    next(fillers[0])
                    budget -= 1
                except StopIteration:
                    fillers.pop(0)

        # prologue: projections for batch 0 (PE-only ramp); batch 1's
        # x-loads and projection generator queue up behind it
        prefetch_x(0)
        prefetch_x(1)
        for _ in gen_proj(0):
            pass
        fillers.append(gen_proj(1))

        # ---- flattened software-pipelined attention stream ----
        # one pair-step = 2 key tiles x 2 heads: 2x(QK pair + exp), then the
        # (a few steps earlier, per pv_sched) DoubleRow PV pair.  Chunks of
        # 512 queries flow back-to-back so the Scalar engine never stalls at
        # chunk borders.
        chunks = [(b, i4) for b in range(B) for i4 in range(NT4)]
        S = len(chunks) * NP
        pvs = {}     # chunk idx -> (pv0, pv1)
        expts = {}   # (chunk, pair) -> epair tile

        # pv pair -> emission step: finish each chunk's PV one step into the
        # next chunk so the normalize chain gets ~2 steps before the PSUM
        # banks are rewritten; chunk 0 lags more (v1 is still being built).
        from collections import defaultdict
        pv_sched = defaultdict(list)
        for c in range(len(chunks)):
            offs = [2, 3, 4, 5, 6, 7, 8, 9]
            for p, off in enumerate(offs):
                pv_sched[8 * c + off].append((c, p))
        S_END = max(pv_sched) + 1

        def chunk_start(ci):
            b, i4 = chunks[ci]
            if i4 == 0:
                tiles[b]["ctxT"] = bigs.tile([F, T], MM_DT, tag="ctxT",
                                             name=f"ctxT{b}")
            # queue the next batch's x loads (spread across pulls) and
            # projections at the start of this batch's attention;
            # batch 1 was queued by the prologue
            if i4 == 0 and 1 < b + 1 < B:
                fillers.append(gen_xt(b + 1))
                fillers.append(gen_proj(b + 1))

        def pv_step(ci, pp):
            b, i4 = chunks[ci]
            v1 = tiles[b]["v1"]
            if pp == 0:
                pvs[ci] = (ps_pv.tile([128, TILE_N], FP32, tag="pv0",
                                      name=f"pv0_{ci}"),
                           ps_pv.tile([128, TILE_N], FP32, tag="pv1",
                                      name=f"pv1_{ci}"))
            e = expts.pop((ci, pp))
            for h, pv in ((0, pvs[ci][0]), (1, pvs[ci][1])):
                nc.tensor.matmul(
                    pv[:], v1[:, 2 * pp : 2 * pp + 2, h, :],
                    e[:, :, h, :], start=(pp == 0), stop=(pp == NP - 1),
                    perf_mode=DR)
            if pp == NP - 1:
                chunk_finish(ci)

        def chunk_finish(ci):
            b, i4 = chunks[ci]
            isl = slice(i4 * TILE_N, (i4 + 1) * TILE_N)
            ctxT = tiles[b]["ctxT"]
            pv0, pv1 = pvs.pop(ci)
            # psum rows: 0-63 ctx_hi, 64 denom, 65-127 lo-correction.
            # One fast copy per head releases the PSUM bank for the next
            # chunk; the whole normalize chain then runs from SBUF.
            Ps = []
            for h, pv in ((0, pv0), (1, pv1)):
                P = small.tile([128, TILE_N], FP32, tag=f"P{h}")
                nc.vector.tensor_copy(P[:], pv[:])
                Ps.append(P)
            rd = small.tile([1, HPC, TILE_N], FP32, tag="rd")
            dnv = small.tile([1, HPC, TILE_N], FP32, tag="dnv")
            nc.vector.tensor_copy(dnv[:, 0, :], Ps[0][D : D + 1, :])
            nc.vector.tensor_copy(dnv[:, 1, :], Ps[1][D : D + 1, :])
            nc.vector.reciprocal_approx_fast(rd[:], dnv[:])
            bc = small.tile([D, HPC, TILE_N], FP32, tag="bc")
            nc.gpsimd.partition_broadcast(bc[:], rd[:])
            for h, P in ((0, Ps[0]), (1, Ps[1])):
                # shift lo (rows 65..127) down one partition via DMA
                # (engines cannot read unaligned partition bases);
                # lo64 row 63 stays zero (dim 63 is hi-only)
                nc.sync.dma_start(lo64[h][0:NLO, :], P[D + 1 : 128, :])
                hs = small.tile([D, TILE_N], FP32, tag="hs")
                nc.vector.tensor_add(hs[:], P[0:D, :], lo64[h][:])
                nc.vector.tensor_mul(
                    ctxT[h * D : (h + 1) * D, isl], hs[:], bc[:, h, :])
            fillers.append(gen_wo_t4(b, i4))

        started = set()
        for s in range(S_END):
            due = pv_sched.get(s, [])
            if s < S:
                ci, p = divmod(s, NP)
                b, i4 = chunks[ci]
                if p == 0:
                    chunk_start(ci)
                qT, kT = tiles[b]["qT"], tiles[b]["kT"]
                isl = slice(i4 * TILE_N, (i4 + 1) * TILE_N)
                epair = epool.tile([128, 2, HPC, TILE_N], PV_DT, tag="expt")
                expts[ci, p] = epair
                for j2 in range(2):
                    j = 2 * p + j2
                    jsl = slice(j * TILE_K, (j + 1) * TILE_K)
                    qk = ps_qk.tile([128, HPC, TILE_N], FP32, tag="qk")
                    # heads in distinct PE row-groups -> run concurrently
                    nc.tensor.matmul(qk[:, 0, :], kT[0:D, jsl],
                                     qT[0:D, isl], start=True, stop=True)
                    nc.tensor.matmul(qk[:, 1, :],
                                     kT[D : 2 * D, jsl], qT[D : 2 * D, isl],
                                     start=True, stop=True)
                    nc.scalar.activation(epair[:, j2], qk[:], Exp,
                                         bias=nln16[:])
                    # keep the in-order PE stream fed while ACT runs exp
                    if j2 == 0:
                        for c2, p2 in due[0:1]:
                            pv_step(c2, p2)
                    else:
                        for c2, p2 in due[1:]:
                            pv_step(c2, p2)
                    pull(2)
            else:
                for c2, p2 in due:
                    pv_step(c2, p2)

        # drain remaining fillers (last batch's final wo chunks)
        pull(10 ** 9)


_CACHE = {}


def _get_nc():
    if "nc" not in _CACHE:
        nc = bacc.Bacc("TRN2", target_bir_lowering=False, debug=False,
                       num_devices=NCORES)
        with tile.TileContext(nc) as tc:
            build_kernel_body(tc)
        nc.compile()
        _CACHE["nc"] = nc
    return _CACHE["nc"]


def host_prep(x, Wq, bq, Wk, bk, Wv, bv, Wo, bo):
    f16 = np.float16
    x = np.asarray(x, np.float32)
    xT = np.ascontiguousarray(x.reshape(TT, C).T.astype(f16))
    scale = np.float32(1.0 / np.sqrt(D))
    in_maps = []
    for c in range(NCORES):
        fsl = slice(c * F, (c + 1) * F)
        in_maps.append({
            "xT": xT,
            "wq": np.ascontiguousarray(
                (np.asarray(Wq, np.float32)[:, fsl] * scale).astype(f16)),
            "wk": np.ascontiguousarray(np.asarray(Wk, np.float32)[:, fsl].astype(f16)),
            "wv": np.ascontiguousarray(np.asarray(Wv, np.float32)[:, fsl].astype(f16)),
            "wo": np.ascontiguousarray(np.asarray(Wo, np.float32)[fsl, :].astype(f16)),
            "bq": np.ascontiguousarray(np.asarray(bq, np.float32)[fsl] * scale),
            "bk": np.ascontiguousarray(np.asarray(bk, np.float32)[fsl]),
        })
    return in_maps


def host_gather(results, Wo, bo, bv):
    total = np.zeros((C, TT), np.float32)
    for c in range(NCORES):
        total += results[c]["outT"].astype(np.float32)
    out = total.T
    out = out + (np.asarray(bo, np.float32)
                 + np.asarray(bv, np.float32) @ np.asarray(Wo, np.float32))
    return out.reshape(B, T, C)


def _install_profile_hook():
    """Make trace=True work under axon when antenv.axon_hooks is absent."""
    import sys
    import types

    try:
        import antenv.axon_hooks  # noqa: F401
        return
    except ImportError:
        pass
    import antenv
    from trn_agent_boot.trn_boot import _ntff_profile_via_ctypes

    mod = types.ModuleType("antenv.axon_hooks")
    holder = [None]
    mod.set_axon_ntff_profile_hook = lambda h: holder.__setitem__(0, h)
    mod.get_axon_ntff_profile_hook = lambda: holder[0]
    sys.modules["antenv.axon_hooks"] = mod
    antenv.axon_hooks = mod
    mod.set_axon_ntff_profile_hook(
        _ntff_profile_via_ctypes("/opt/axon/libaxon_pjrt.so")
    )
    # artifact upload needs internal storage; keep profiles local
    import concourse.bass_utils as bu
    bu.upload_artifacts = lambda tmpdir: f"local:{tmpdir}"


def kernel(x, Wq, bq, Wk, bk, Wv, bv, Wo, bo, _trace=False):
    if _trace:
        _install_profile_hook()
    nc = _get_nc()
    in_maps = host_prep(x, Wq, bq, Wk, bk, Wv, bv, Wo, bo)
    res = run_bass_kernel_spmd(nc, in_maps, core_ids=list(range(NCORES)),
                               trace=_trace)
    _CACHE["last_result"] = res
    return host_gather(res.results, Wo, bo, bv)
